# revision 1
# baseline (speedup 1.0000x reference)
"""Trainium2 Bass kernel for the double additive-attention block.

reference:
  scores_a = relu(emb @ W_a1.T + g @ W_a2.T) @ v_a          # per batch, [N]
  a        = softmax(scores_a)                               # over N
  c        = sum_n a_n * emb[n]                              # [E]
  scores_o = relu(emb @ W_o1.T + c @ W_o2.T) @ v_o
  out      = softmax(scores_o + mask)                        # over N

Sharding: data-parallel over batch B=32 -> 4 batches on each of 8 cores.
Params are tiny and replicated. All softmax axes local per core.

Per-batch on-device layout (blocked): partition p holds tokens
[p*64, p*64+64), i.e. token n lives at [p=n//64, t=n%64] of a [128, 64]
tile. The natural-layout embedding tile is [128, 64, 128] (p, t, e).

Data plane is float16: emb is cast fp32->fp16 during the HBM DMA
(SWDGE), the PE transposes nat->embT in fp16 (1 cyc/row vs 2 for fp32),
PSUM->SBUF evictions move packed fp16 at the DVE 2x rate, and all big
matmuls run with fp16 operands (fp32 PSUM accumulate). Scores, softmax
and biases stay fp32. The attention weights for the context matmul are
normalized (a = exp*recip(sum)) before the fp16 cast so they fit fp16
range without a max-subtraction pass.
"""

import os
import sys
from contextlib import ExitStack

import numpy as np

if "/opt/trn_rl_repo" not in sys.path:
    sys.path.insert(0, "/opt/trn_rl_repo")
os.environ.setdefault("MYCRO_LOCAL_CACHE", "1")

import concourse.bass as bass
import concourse.tile as tile
from concourse import mybir
from concourse import bass_isa
from concourse.bass_utils import run_bass_kernel_spmd

B, N, E, A = 32, 8192, 128, 128
NCORES = 8
BPC = B // NCORES          # batches per core
NT = N // 128              # 64 column-tiles of the [128, 64] score layout
CH = 512                   # moving free-dim per big matmul (1 PSUM bank fp32)
NCH = N // CH              # 16 chunks per pass -> 16 vdot slots in one pack
TGRP = 8                   # transposes per PSUM eviction group (fp16)
F32 = mybir.dt.float32

MM_DT_NAME = os.environ.get("KERNEL_MM_DT", "float16")
MM_DT = getattr(mybir.dt, MM_DT_NAME)
# Engine assignment patterns (cyclic): 'A' = ACT, 'D' = DVE.
# relu: ACT activation(Relu, bias) vs DVE tensor_scalar(add, max) per chunk.
RELU_PAT_A = os.environ.get("KERNEL_RELU_PAT_A", "DAAD")
RELU_PAT_O = os.environ.get("KERNEL_RELU_PAT_O", "DAAD")
# embT eviction groups (of TGRP transposes each)
EVICT_PAT = os.environ.get("KERNEL_EVICT_PAT", "DDA")
# engine for the [16, CH] score-rows eviction
ROWS_ENG = os.environ.get("KERNEL_ROWS_ENG", "A")
# how many main matmuls run ahead of the v-dot consuming their relu output
LOOKAHEAD = int(os.environ.get("KERNEL_LOOKAHEAD", "3"))
# v-dot accumulation packs per pass (1 = one [16,CH] pack, 2 = two [8,CH])
NPACKS = int(os.environ.get("KERNEL_NPACKS", "1"))
# 2 = run v-dots on two PE column-group tiles (even/odd chunks) so they
# overlap in the array; 1 = single tile. HW-only win (cost model is
# tile-concurrency blind).
VDOT_CT = int(os.environ.get("KERNEL_VDOT_CT", "1"))
# 1 = one relu instruction per TWO chunks ([A, 2*CH] px tiles spanning two
# PSUM banks, pp_big bufs halved) - amortizes ACT/DVE fixed overheads
RELU_PAIR = int(os.environ.get("KERNEL_RELU_PAIR", "0"))
# 1 = softmax total via gpsimd partition_all_reduce (Pool engine) instead of
# the PE-matmul + broadcast chain
GPSM = int(os.environ.get("KERNEL_GPSM", "0"))

RELU = mybir.ActivationFunctionType.Relu
EXP = mybir.ActivationFunctionType.Exp
COPY = mybir.ActivationFunctionType.Copy
MAX = mybir.AluOpType.max
ADD = mybir.AluOpType.add
AX_X = mybir.AxisListType.X


def build(mm_dt, iters=1, ablate=(), nbatch=None):
    nc = bass.Bass(target_bir_lowering=False)
    dt_sz = mybir.dt.size(mm_dt)
    tgrp = TGRP if dt_sz == 2 else 4   # keep eviction group inside one bank

    emb = nc.dram_tensor("emb", [BPC, N, E], F32, kind="ExternalInput")
    mask = nc.dram_tensor("mask", [BPC, N], F32, kind="ExternalInput")
    # consts packed per dtype so each needs a single DMA (single sem lane).
    # cf (fp32): gT(4) | ident(128) | w2a(128) | w2o(128)
    cf = nc.dram_tensor("cf", [128, 4 + 128 + 2 * A], F32, kind="ExternalInput")
    # ch (mm_dt): identr(128) | w1a(128) | w1o(128) | va16(256) | vo16(256)
    ch = nc.dram_tensor("ch", [128, 3 * 128 + 2 * 256], mm_dt, kind="ExternalInput")
    out = nc.dram_tensor("out", [BPC, N], F32, kind="ExternalOutput")

    # blocked views: n = p*NT + t
    emb_r = emb.rearrange("b (p t) e -> b p t e", p=128)
    mask_r = mask.rearrange("b (p t) -> b p t", p=128)
    out_r = out.rearrange("b (p t) -> b p t", p=128)

    with tile.TileContext(nc) as tc, ExitStack() as ctx:
        consts = ctx.enter_context(tc.tile_pool(name="consts", bufs=1))
        big = ctx.enter_context(tc.tile_pool(
            name="big", bufs=int(os.environ.get("KERNEL_BIG_BUFS", "3"))))
        work = ctx.enter_context(tc.tile_pool(name="work", bufs=8))
        small = ctx.enter_context(tc.tile_pool(
            name="small", bufs=int(os.environ.get("KERNEL_SMALL_BUFS", "2"))))
        maskp = ctx.enter_context(tc.tile_pool(name="maskp", bufs=3))
        pp_t = ctx.enter_context(tc.tile_pool(
            name="pp_t", bufs=int(os.environ.get("KERNEL_PT_BUFS", "1")),
            space="PSUM"))
        pp_big = ctx.enter_context(
            tc.tile_pool(name="pp_big", bufs=2 if RELU_PAIR else LOOKAHEAD + 1,
                         space="PSUM"))
        pp_row = ctx.enter_context(tc.tile_pool(name="pp_row", bufs=1, space="PSUM"))
        pp_sc = ctx.enter_context(tc.tile_pool(name="pp_sc", bufs=1, space="PSUM"))
        pp_misc = ctx.enter_context(tc.tile_pool(name="pp_misc", bufs=1, space="PSUM"))

        cf_sb = consts.tile([128, 4 + 128 + 2 * A], F32, tag="cf")
        nc.sync.dma_start(out=cf_sb, in_=cf[:])
        ch_sb = consts.tile([128, 3 * 128 + 2 * 256], mm_dt, tag="ch")
        nc.sync.dma_start(out=ch_sb, in_=ch[:])

        gT_sb = cf_sb[:, 0:4]
        ident_sb = cf_sb[:, 4:132]
        w2a_sb = cf_sb[:, 132:260]
        w2o_sb = cf_sb[:, 260:388]
        identr_sb = ch_sb[:, 0:128]
        w1a_sb = ch_sb[:, 128:256]
        w1o_sb = ch_sb[:, 256:384]
        va16_sb = ch_sb[:, 384:640].rearrange("a (c k) -> a c k", c=16)
        vo16_sb = ch_sb[:, 640:896].rearrange("a (c k) -> a c k", c=16)

        ones_row = consts.tile([1, 128], F32, tag="ones_row")
        nc.vector.memset(ones_row, 1.0)
        ones_col = consts.tile([128, 1], F32, tag="ones_col")
        nc.vector.memset(ones_col, 1.0)

        # dummy matmul so the PE observes the ch-DMA lane once, up front;
        # walrus allows only one sync wait per (self-loading) Matmult.
        pdum = pp_misc.tile([128, 4], F32, tag="m")
        nc.tensor.matmul(pdum, lhsT=identr_sb, rhs=identr_sb[:, 0:4],
                         start=True, stop=True)
        # bias_a for all local batches: [A, BPC] = W_a2 @ g.T
        pba = pp_misc.tile([A, BPC], F32, tag="m")
        nc.tensor.matmul(pba, lhsT=w2a_sb, rhs=gT_sb, start=True, stop=True)
        ba_sb = consts.tile([A, BPC], F32, tag="ba")
        nc.vector.tensor_copy(out=ba_sb, in_=pba)

        def bcast_scalar(src11, tag):
            """[1,1] sbuf scalar -> [128,1] sbuf per-partition vector."""
            pb = pp_misc.tile([128, 1], F32, tag="m")
            nc.tensor.matmul(pb, lhsT=ones_row, rhs=src11, start=True, stop=True)
            dst = small.tile([128, 1], F32, tag=tag)
            nc.vector.tensor_copy(out=dst, in_=pb)
            return dst

        def score_pass(embT, w1_sb, v16_sb, bias_ap, pat):
            # Returns psc: PSUM [128, NT]; col T=4c+q holds scores of embT
            # col 512c + 128q + p  (== token n = 64p + t, t = 4c+q).
            # The main matmul runs LOOKAHEAD chunks ahead of the v-dot so
            # the PE never stalls on the ACT/DVE relu of the current chunk.
            # psc shares the pp_big ring (allocated after the chunk loop so
            # the ring slots recycle cleanly).
            psc = pp_sc.tile([128, NT], F32, tag="psc")
            srelu = {}
            prow = None

            if VDOT_CT == 2:
                # strip t holds chunks {2j+t}: psc col T = 4(2j+t)+q
                pscv = psc.rearrange("p (j t q) -> p j t q", t=2, q=4)
            else:
                pscv = psc.rearrange("p (pk j q) -> p pk j q", pk=NPACKS, q=4)
            w = 8 if VDOT_CT == 2 else NCH // NPACKS

            def close_pack(pk, rows_src):
                rows_sb = work.tile([w, CH], F32, tag="rows")
                if ROWS_ENG == "A":
                    nc.scalar.activation(out=rows_sb, in_=rows_src, func=COPY)
                else:
                    nc.vector.tensor_copy(out=rows_sb, in_=rows_src)
                for q in range(4 if "tb" not in ablate else 0):
                    dst = (pscv[:, :, pk, q] if VDOT_CT == 2
                           else pscv[:, pk, :, q])
                    nc.tensor.transpose(
                        dst,
                        rows_sb[:, q * 128:(q + 1) * 128],
                        ident_sb[0:w, 0:w],
                    )

            la = 2 * (1 + (LOOKAHEAD > 2)) if RELU_PAIR else LOOKAHEAD
            for c in range(NCH + la):
                if c < NCH and RELU_PAIR:
                    if c % 2 == 0:
                        pxp = pp_big.tile([A, 2 * CH], F32, tag="big")
                    px = pxp[:, (c % 2) * CH:(c % 2 + 1) * CH]
                    if "w" not in ablate:
                        nc.tensor.matmul(
                            px,
                            lhsT=w1_sb,
                            rhs=embT[:, c * CH:(c + 1) * CH],
                            start=True, stop=True,
                        )
                    if c % 2 == 1 and "relu" not in ablate:
                        sr = work.tile([A, 2 * CH], mm_dt, tag="srelu")
                        srelu[c - 1] = sr[:, 0:CH]
                        srelu[c] = sr[:, CH:2 * CH]
                        if pat[(c // 2) % len(pat)] == "D":
                            nc.vector.tensor_scalar(
                                sr, pxp, bias_ap, 0.0, ADD, MAX)
                        else:
                            nc.scalar.activation(out=sr, in_=pxp,
                                                 func=RELU, bias=bias_ap,
                                                 scale=1.0)
                elif c < NCH:
                    px = pp_big.tile([A, CH], F32, tag="big")
                    if "w" not in ablate:
                        nc.tensor.matmul(
                            px,
                            lhsT=w1_sb,
                            rhs=embT[:, c * CH:(c + 1) * CH],
                            start=True, stop=True,
                        )
                    sr = work.tile([A, CH], mm_dt, tag="srelu")
                    srelu[c] = sr
                    if "relu" not in ablate:
                        if pat[c % len(pat)] == "D":
                            nc.vector.tensor_scalar(
                                sr, px, bias_ap, 0.0, ADD, MAX)
                        else:
                            nc.scalar.activation(out=sr, in_=px,
                                                 func=RELU, bias=bias_ap,
                                                 scale=1.0)
                cc = c - la
                if 0 <= cc < NCH and "v" not in ablate:
                    if VDOT_CT == 2:
                        # even/odd chunks on separate PE column-group tiles
                        # (partition bases 0/64) so consecutive v-dots
                        # overlap inside the array.
                        t, j = cc % 2, cc // 2
                        if cc == 0:
                            prow = pp_row.tile([128, CH], F32, tag="row16")
                        lo = cc - j   # 8-window of the [A,16] plane w/ v at j
                        nc.tensor.matmul(
                            prow[64 * t:64 * t + 8, :],
                            lhsT=v16_sb[:, cc, lo:lo + 8],
                            rhs=srelu.pop(cc),
                            start=(j == 0), stop=(j == 7),
                            tile_position=(0, 64 * t),
                            skip_group_check=True,
                        )
                        if cc >= NCH - 2:
                            close_pack(t, prow[64 * t:64 * t + 8, :])
                    else:
                        if cc % w == 0:
                            prow = pp_row.tile([w, CH], F32, tag="row16")
                        # v-dot: stationary [A,w] with column cc%w = v
                        # -> accumulates chunk cc's scores into row cc%w.
                        nc.tensor.matmul(
                            prow,
                            lhsT=v16_sb[:, cc, (cc // w) * w:(cc // w + 1) * w],
                            rhs=srelu.pop(cc),
                            start=(cc % w == 0), stop=(cc % w == w - 1),
                        )
                        if cc % w == w - 1:
                            close_pack(cc // w, prow)
            return psc

        def softmax_stats(sc_ap, tag):
            """Scores are bounded (|s| < ~60 << 88) so fp32 exp without
            max-subtraction is safe; skipping the global max removes six
            serial cross-engine hops per softmax.

            Returns (pexp [128,NT] fp32 sbuf unnormalized exp, recb
            [128,1] sbuf broadcast reciprocal of the global sum)."""
            pexp = work.tile([128, NT], F32, tag="pexp_" + tag)
            rowsum = small.tile([128, 1], F32, tag="rowsum_" + tag)
            nc.scalar.activation(out=pexp, in_=sc_ap, func=EXP,
                                 bias=0.0, scale=1.0, accum_out=rowsum)
            if GPSM:
                # cross-partition sum on the (otherwise idle) Pool engine;
                # the all-reduce output is already broadcast per-partition.
                tot128 = small.tile([128, 1], F32, tag="tot_" + tag)
                nc.gpsimd.partition_all_reduce(tot128, rowsum, channels=128,
                                               reduce_op=bass_isa.ReduceOp.add)
                recb = small.tile([128, 1], F32, tag="recb_" + tag)
                nc.vector.reciprocal(recb, tot128)
                return pexp, recb
            ptot = pp_misc.tile([1, 1], F32, tag="m")
            nc.tensor.matmul(ptot, lhsT=rowsum, rhs=ones_col,
                             start=True, stop=True)
            rec = small.tile([1, 1], F32, tag="rec_" + tag)
            nc.vector.reciprocal(rec, ptot)
            recb = bcast_scalar(rec, "recb_" + tag)
            return pexp, recb

        nb = nbatch or BPC
        state = [dict() for _ in range(nb)]

        def phase_load(b):
            st = state[b]
            nat = big.tile([128, NT, E], mm_dt, tag="nat")
            st["nat"] = nat
            # ~1MB(fp32-read) casting loads via SWDGE; each chunk stays
            # above the ~1MB DMA line-rate knee on the read side. Batch 0
            # uses finer chunks so the first transposes start sooner.
            nch = int(os.environ.get("KERNEL_NCH0", "8")) if b == 0 else int(os.environ.get("KERNEL_NCHB", "4"))
            q = NT // nch
            for h in range(nch):
                nc.gpsimd.dma_start(out=nat[:, h * q:(h + 1) * q, :],
                                    in_=emb_r[b][:, h * q:(h + 1) * q, :])
            mask_sb = maskp.tile([128, NT], F32, tag="mask")
            st["mask"] = mask_sb
            nc.sync.dma_start(out=mask_sb, in_=mask_r[b])

        def phase_trans(b):
            st = state[b]
            nat = st["nat"]
            pdmb = pp_misc.tile([128, 4], F32, tag="m")
            nc.tensor.matmul(pdmb, lhsT=nat[:, 0, :], rhs=nat[:, 0, 0:4],
                             start=True, stop=True)
            embT = big.tile([E, N], mm_dt, tag="embT")
            st["embT"] = embT
            ngr = (NT // tgrp) if "tr" not in ablate else 0
            for g in range(ngr):
                pt = pp_t.tile([128, tgrp * 128], mm_dt, tag="pt")
                for j in range(tgrp):
                    nc.tensor.transpose(pt[:, j * 128:(j + 1) * 128],
                                        nat[:, g * tgrp + j, :], identr_sb)
                dst = embT[:, g * tgrp * 128:(g + 1) * tgrp * 128]
                if EVICT_PAT[g % len(EVICT_PAT)] == "A":
                    nc.scalar.activation(out=dst, in_=pt, func=COPY)
                else:
                    nc.vector.tensor_copy(out=dst, in_=pt)

        def phase_passA(b):
            st = state[b]
            st["psc"] = score_pass(st["embT"], w1a_sb, va16_sb,
                                   ba_sb[:, b:b + 1], RELU_PAT_A)

        def phase_softA(b):
            st = state[b]
            pexp, recb = softmax_stats(st["psc"], "a")
            # normalized fp16 attention weights for the fp16 ctx matmul
            a16 = work.tile([128, NT], MM_DT, tag="a16")
            st["a16"] = a16
            nc.vector.tensor_scalar_mul(a16, pexp, recb)

        def phase_ctx(b):
            st = state[b]
            nat, a16 = st["nat"], st["a16"]
            # context: c[e] = sum_n a_n * emb[n, e].  Grouped 4 tiles per
            # matmul so the moving free dim is 512; the diagonal blocks of
            # the [4, 512] accumulator hold the true partial contexts.
            pc4 = pp_misc.tile([4, 4 * E], F32, tag="m")
            ng = NT // 4 if "ctx" not in ablate else 1
            for g in range(ng):
                nc.tensor.matmul(
                    pc4,
                    lhsT=a16[:, 4 * g:4 * g + 4],
                    rhs=nat[:, 4 * g:4 * g + 4, :].rearrange("p t e -> p (t e)"),
                    start=(g == 0), stop=(g == ng - 1),
                )
            # Accumulator rows live at partitions 0-3; partition-sliced reads
            # must start at 0/32/64/96, so: copy rows to SBUF, PE-transpose
            # each 128-block, pick the diagonal via a stride-5 free-axis AP,
            # reduce into cT [E, 1].
            cd4 = work.tile([4, 4 * E], F32, tag="cd4")
            nc.vector.tensor_copy(out=cd4, in_=pc4)
            ptd = pp_misc.tile([128, 4, 4], F32, tag="m")
            for q in range(4):
                nc.tensor.transpose(ptd[:, q, :], cd4[0:4, q * E:(q + 1) * E],
                                    ident_sb[0:4, 0:4])
            ptd_flat = ptd.rearrange("p a b -> p (a b)")
            diag = bass.AP(tensor=ptd_flat.tensor, offset=ptd_flat.offset,
                           ap=[ptd_flat.ap[0], [5, 4]])
            cT = small.tile([E, 1], F32, tag="cT")
            nc.vector.tensor_reduce(cT, diag, axis=AX_X, op=ADD)
            # bias_o = W_o2 @ c  (a16 already normalized)
            pbo = pp_misc.tile([A, 1], F32, tag="m")
            nc.tensor.matmul(pbo, lhsT=w2o_sb, rhs=cT, start=True, stop=True)
            bo = small.tile([A, 1], F32, tag="bo")
            st["bo"] = bo
            nc.vector.tensor_copy(out=bo, in_=pbo)

        def phase_passO(b):
            st = state[b]
            st["psc2"] = score_pass(st["embT"], w1o_sb, vo16_sb, st["bo"],
                                    RELU_PAT_O)

        def phase_softO(b):
            st = state[b]
            sc2 = work.tile([128, NT], F32, tag="sc2")
            nc.vector.tensor_add(sc2, st["psc2"], st["mask"])
            pexp2, recb2 = softmax_stats(sc2, "o")
            outt = work.tile([128, NT], F32, tag="outt")
            nc.vector.tensor_scalar_mul(outt, pexp2, recb2)
            nc.sync.dma_start(out=out_r[b], in_=outt)

        PIPE = os.environ.get("KERNEL_PIPE", "3")
        for _ in range(iters):
            if PIPE == "0":
                for b in range(nb):
                    phase_load(b)
                    phase_trans(b)
                    phase_passA(b)
                    phase_softA(b)
                    phase_ctx(b)
                    phase_passO(b)
                    phase_softO(b)
            elif PIPE == "3":
                phase_load(0)
                phase_trans(0)
                for b in range(nb):
                    if b + 1 < nb:
                        phase_load(b + 1)
                    phase_passA(b)
                    phase_softA(b)
                    phase_ctx(b)
                    phase_passO(b)
                    if b + 1 < nb:
                        phase_trans(b + 1)
                    phase_softO(b)
            elif PIPE == "4":
                # two batches in flight: batch b+1's transpose + pass A are
                # emitted between batch b's ctx and softO so the PE (and the
                # relu engines) stay fed across the serial softmax chains.
                phase_load(0)
                phase_trans(0)
                phase_passA(0)
                for b in range(nb):
                    if b + 1 < nb:
                        phase_load(b + 1)
                    phase_softA(b)
                    phase_ctx(b)
                    if b + 1 < nb:
                        phase_trans(b + 1)
                    phase_passO(b)
                    if b + 1 < nb:
                        phase_passA(b + 1)
                    phase_softO(b)

    return nc


def _fix_multiwait(bir):
    """walrus's PE Matmult codegen accepts a single sync wait. Hoist extra
    waits onto wait-only EventSemaphore instructions inserted just before."""
    n = 0
    for fn in bir["functions"]:
        for bb in fn["blocks"]:
            new = []
            for inst in bb["instructions"]:
                si = inst.get("sync_info") or {}
                w = si.get("on_wait") or []
                if len(w) > 1:
                    for extra in w[:-1]:
                        n += 1
                        new.append({
                            "debug": inst.get("debug", 0),
                            "engine": inst["engine"],
                            "ins": [], "outs": [],
                            "name": f"{inst['name']}-prewait{n}",
                            "opcode": "EventSemaphore",
                            "sync_info": {"on_update": [], "on_wait": [extra]},
                        })
                    si["on_wait"] = [w[-1]]
                new.append(inst)
            bb["instructions"] = new
    return bir


def _patch_serialization(nc):
    import orjson

    orig = nc.to_json_bytes

    def patched(*a, **kw):
        return orjson.dumps(_fix_multiwait(orjson.loads(orig(*a, **kw))))

    nc.to_json_bytes = patched
    return nc


_NC_CACHE = {}


def _get_nc(mm_dt_name=MM_DT_NAME, iters=1):
    key = (mm_dt_name, iters)
    if key not in _NC_CACHE:
        _NC_CACHE[key] = _patch_serialization(
            build(getattr(mybir.dt, mm_dt_name), iters=iters))
    return _NC_CACHE[key]


def _vz16(v):
    z = np.zeros((A, 16, 16), np.float32)
    for c in range(16):
        z[:, c, c] = v
    return z


def _prep_in_maps(inputs, mm_np=None):
    mm_np = mm_np or mybir.dt.np(MM_DT)
    embeddings = np.ascontiguousarray(np.asarray(inputs["embeddings"], np.float32))
    gru = np.asarray(inputs["gru_output"], np.float32).reshape(B, E)
    mask = np.ascontiguousarray(np.asarray(inputs["action_mask"], np.float32))
    W_a = np.asarray(inputs["W_a"], np.float32)
    W_o = np.asarray(inputs["W_o"], np.float32)
    v_a = np.asarray(inputs["v_a"], np.float32)
    v_o = np.asarray(inputs["v_o"], np.float32)

    eye = np.eye(128, dtype=np.float32)
    ch = np.concatenate(
        [eye, W_a[:, :E].T, W_o[:, :E].T,
         _vz16(v_a).reshape(A, 256), _vz16(v_o).reshape(A, 256)], axis=1)
    ch = np.ascontiguousarray(ch).astype(mm_np)

    in_maps = []
    for c in range(NCORES):
        sl = slice(c * BPC, (c + 1) * BPC)
        cf = np.concatenate(
            [gru[sl].T, eye, W_a[:, E:].T, W_o[:, E:].T], axis=1)
        in_maps.append({
            "emb": embeddings[sl],
            "mask": mask[sl],
            "cf": np.ascontiguousarray(cf, np.float32),
            "ch": ch,
        })
    return in_maps


def run(inputs, trace=False):
    nc = _get_nc()
    in_maps = _prep_in_maps(inputs)
    res = run_bass_kernel_spmd(nc, in_maps, core_ids=list(range(NCORES)),
                               trace=trace)
    out = np.concatenate([res.results[c]["out"] for c in range(NCORES)], axis=0)
    return out.reshape(B, N), res


def kernel(**inputs):
    out, _ = run(inputs, trace=False)
    return out


def make_runner(mm_dt_name=MM_DT_NAME, iters=1):
    """Build the sharded PJRT callable once, for repeated timed execution.

    Mirrors the multi-core branch of bass2jax.run_bass_via_pjrt."""
    import jax
    from jax.experimental.shard_map import shard_map
    from jax.sharding import Mesh, PartitionSpec

    from concourse import bass2jax as b2j
    from concourse import mybir as _mybir

    b2j.install_neuronx_cc_hook()
    nc = _get_nc(mm_dt_name, iters=iters)

    partition_name = (nc.partition_id_tensor.name
                      if nc.partition_id_tensor else None)
    in_names, out_names, out_avals, zero_outs = [], [], [], []
    for alloc in nc.m.functions[0].allocations:
        if not isinstance(alloc, _mybir.MemoryLocationSet):
            continue
        name = alloc.memorylocations[0].name
        if alloc.kind == "ExternalInput":
            if name != partition_name:
                in_names.append(name)
        elif alloc.kind == "ExternalOutput":
            out_names.append(name)
            shape = tuple(alloc.tensor_shape)
            dtype = _mybir.dt.np(alloc.dtype)
            out_avals.append(jax.core.ShapedArray(shape, dtype))
            zero_outs.append(np.zeros(shape, dtype))
    n_params = len(in_names)
    n_outs = len(out_avals)
    all_names = in_names + out_names
    if partition_name is not None:
        all_names = all_names + [partition_name]

    def _body(*args):
        operands = list(args)
        if partition_name is not None:
            operands.append(b2j.partition_id_tensor())
        outs = b2j._bass_exec_p.bind(
            *operands,
            out_avals=tuple(out_avals),
            in_names=tuple(all_names),
            out_names=tuple(out_names),
            lowering_input_output_aliases=(),
            sim_require_finite=True,
            sim_require_nnan=True,
            nc=nc,
        )
        return tuple(outs)

    devices = jax.devices()[:NCORES]
    mesh = Mesh(np.asarray(devices), ("core",))
    donate = tuple(range(n_params, n_params + n_outs))
    sharded = jax.jit(
        shard_map(_body, mesh=mesh,
                  in_specs=(PartitionSpec("core"),) * (n_params + n_outs),
                  out_specs=(PartitionSpec("core"),) * n_outs,
                  check_rep=False),
        donate_argnums=donate, keep_unused=True,
    )

    def runner(inputs, iters=10, burst=True):
        import time as _time
        in_maps = _prep_in_maps(inputs, mm_np=_mybir.dt.np(
            getattr(_mybir.dt, mm_dt_name)))
        concat_in = [
            np.concatenate([np.asarray(in_maps[c][nm]) for c in range(NCORES)], axis=0)
            for nm in in_names
        ]
        concat_in = [jax.device_put(x) for x in concat_in]
        for x in concat_in:
            x.block_until_ready()

        def zeros():
            return [np.zeros((NCORES * z.shape[0], *z.shape[1:]), z.dtype)
                    for z in zero_outs]

        out = sharded(*concat_in, *zeros())  # warm / compile
        [o.block_until_ready() for o in out]
        result = np.asarray(out[0]).reshape(B, N)

        seq_times = []
        for _ in range(iters):
            zs = zeros()
            t0 = _time.perf_counter()
            out = sharded(*concat_in, *zs)
            [o.block_until_ready() for o in out]
            seq_times.append(_time.perf_counter() - t0)

        zss = [zeros() for _ in range(iters)]
        t0 = _time.perf_counter()
        outs = [sharded(*concat_in, *zs) for zs in zss]
        [o.block_until_ready() for o in outs[-1]]
        burst_time = (_time.perf_counter() - t0) / iters
        return result, {
            "seq_min_s": min(seq_times),
            "seq_med_s": sorted(seq_times)[len(seq_times) // 2],
            "burst_avg_s": burst_time,
        }

    return runner



# revision 4
# speedup vs baseline: 1.0097x; 1.0097x over previous
"""Trainium2 Bass kernel for the double additive-attention block.

reference:
  scores_a = relu(emb @ W_a1.T + g @ W_a2.T) @ v_a          # per batch, [N]
  a        = softmax(scores_a)                               # over N
  c        = sum_n a_n * emb[n]                              # [E]
  scores_o = relu(emb @ W_o1.T + c @ W_o2.T) @ v_o
  out      = softmax(scores_o + mask)                        # over N

Sharding: data-parallel over batch B=32 -> 4 batches on each of 8 cores.
Params are tiny and replicated. All softmax axes local per core.

Per-batch on-device layout (blocked): partition p holds tokens
[p*64, p*64+64), i.e. token n lives at [p=n//64, t=n%64] of a [128, 64]
tile. The natural-layout embedding tile is [128, 64, 128] (p, t, e).

Data plane is float16: emb is cast fp32->fp16 during the HBM DMA
(SWDGE), the PE transposes nat->embT in fp16 (1 cyc/row vs 2 for fp32),
PSUM->SBUF evictions move packed fp16 at the DVE 2x rate, and all big
matmuls run with fp16 operands (fp32 PSUM accumulate). Scores, softmax
and biases stay fp32. The attention weights for the context matmul are
normalized (a = exp*recip(sum)) before the fp16 cast so they fit fp16
range without a max-subtraction pass.
"""

import os
import sys
from contextlib import ExitStack

import numpy as np

if "/opt/trn_rl_repo" not in sys.path:
    sys.path.insert(0, "/opt/trn_rl_repo")
os.environ.setdefault("MYCRO_LOCAL_CACHE", "1")

import concourse.bass as bass
import concourse.tile as tile
from concourse import mybir
from concourse import bass_isa
from concourse.bass_utils import run_bass_kernel_spmd

B, N, E, A = 32, 8192, 128, 128
NCORES = 8
BPC = B // NCORES          # batches per core
NT = N // 128              # 64 column-tiles of the [128, 64] score layout
CH = 512                   # moving free-dim per big matmul (1 PSUM bank fp32)
NCH = N // CH              # 16 chunks per pass -> 16 vdot slots in one pack
TGRP = 8                   # transposes per PSUM eviction group (fp16)
F32 = mybir.dt.float32

MM_DT_NAME = os.environ.get("KERNEL_MM_DT", "float16")
MM_DT = getattr(mybir.dt, MM_DT_NAME)
# Engine assignment patterns (cyclic): 'A' = ACT, 'D' = DVE.
# relu: ACT activation(Relu, bias) vs DVE tensor_scalar(add, max) per chunk.
RELU_PAT_A = os.environ.get("KERNEL_RELU_PAT_A", "DAAD")
RELU_PAT_O = os.environ.get("KERNEL_RELU_PAT_O", "DAAD")
# embT eviction groups (of TGRP transposes each)
EVICT_PAT = os.environ.get("KERNEL_EVICT_PAT", "DDA")
# engine for the [16, CH] score-rows eviction
ROWS_ENG = os.environ.get("KERNEL_ROWS_ENG", "A")
# how many main matmuls run ahead of the v-dot consuming their relu output
LOOKAHEAD = int(os.environ.get("KERNEL_LOOKAHEAD", "3"))
# v-dot accumulation packs per pass (1 = one [16,CH] pack, 2 = two [8,CH])
NPACKS = int(os.environ.get("KERNEL_NPACKS", "1"))
# 2 = run v-dots on two PE column-group tiles (even/odd chunks) so they
# overlap in the array; 1 = single tile. HW-only win (cost model is
# tile-concurrency blind).
VDOT_CT = int(os.environ.get("KERNEL_VDOT_CT", "1"))
# 1 = one relu instruction per TWO chunks ([A, 2*CH] px tiles spanning two
# PSUM banks, pp_big bufs halved) - amortizes ACT/DVE fixed overheads
RELU_PAIR = int(os.environ.get("KERNEL_RELU_PAIR", "0"))
# 1 = softmax total via gpsimd partition_all_reduce (Pool engine) instead of
# the PE-matmul + broadcast chain
GPSM = int(os.environ.get("KERNEL_GPSM", "0"))
# bench-only hooks: comma-separated ablation set; all-engine barrier between
# NEFF iterations so the burst slope measures the serialized (one-shot)
# makespan the harness grades.
ABLATE_ENV = tuple(x for x in os.environ.get("KERNEL_ABLATE", "").split(",") if x)
ITERBAR = int(os.environ.get("KERNEL_ITERBAR", "0"))

RELU = mybir.ActivationFunctionType.Relu
EXP = mybir.ActivationFunctionType.Exp
COPY = mybir.ActivationFunctionType.Copy
MAX = mybir.AluOpType.max
ADD = mybir.AluOpType.add
AX_X = mybir.AxisListType.X


def build(mm_dt, iters=1, ablate=(), nbatch=None):
    ablate = tuple(ablate) + ABLATE_ENV
    nc = bass.Bass(target_bir_lowering=False)
    dt_sz = mybir.dt.size(mm_dt)
    tgrp = TGRP if dt_sz == 2 else 4   # keep eviction group inside one bank

    emb = nc.dram_tensor("emb", [BPC, N, E], F32, kind="ExternalInput")
    mask = nc.dram_tensor("mask", [BPC, N], F32, kind="ExternalInput")
    # consts packed per dtype so each needs a single DMA (single sem lane).
    # cf (fp32): gT(4) | ident(128) | w2a(128) | w2o(128)
    cf = nc.dram_tensor("cf", [128, 4 + 128 + 2 * A], F32, kind="ExternalInput")
    # ch (mm_dt): identr(128) | w1a(128) | w1o(128) | va16(256) | vo16(256)
    ch = nc.dram_tensor("ch", [128, 3 * 128 + 2 * 256], mm_dt, kind="ExternalInput")
    out = nc.dram_tensor("out", [BPC, N], F32, kind="ExternalOutput")

    # blocked views: n = p*NT + t
    emb_r = emb.rearrange("b (p t) e -> b p t e", p=128)
    mask_r = mask.rearrange("b (p t) -> b p t", p=128)
    out_r = out.rearrange("b (p t) -> b p t", p=128)

    with tile.TileContext(nc) as tc, ExitStack() as ctx:
        consts = ctx.enter_context(tc.tile_pool(name="consts", bufs=1))
        big = ctx.enter_context(tc.tile_pool(
            name="big", bufs=int(os.environ.get("KERNEL_BIG_BUFS", "3"))))
        work = ctx.enter_context(tc.tile_pool(name="work", bufs=8))
        small = ctx.enter_context(tc.tile_pool(
            name="small", bufs=int(os.environ.get("KERNEL_SMALL_BUFS", "2"))))
        maskp = ctx.enter_context(tc.tile_pool(name="maskp", bufs=3))
        pp_t = ctx.enter_context(tc.tile_pool(
            name="pp_t", bufs=int(os.environ.get("KERNEL_PT_BUFS", "1")),
            space="PSUM"))
        pp_big = ctx.enter_context(
            tc.tile_pool(name="pp_big", bufs=2 if RELU_PAIR else LOOKAHEAD + 1,
                         space="PSUM"))
        pp_row = ctx.enter_context(tc.tile_pool(name="pp_row", bufs=1, space="PSUM"))
        pp_sc = ctx.enter_context(tc.tile_pool(name="pp_sc", bufs=1, space="PSUM"))
        pp_misc = ctx.enter_context(tc.tile_pool(name="pp_misc", bufs=1, space="PSUM"))

        cf_sb = consts.tile([128, 4 + 128 + 2 * A], F32, tag="cf")
        nc.sync.dma_start(out=cf_sb, in_=cf[:])
        ch_sb = consts.tile([128, 3 * 128 + 2 * 256], mm_dt, tag="ch")
        nc.sync.dma_start(out=ch_sb, in_=ch[:])

        gT_sb = cf_sb[:, 0:4]
        ident_sb = cf_sb[:, 4:132]
        w2a_sb = cf_sb[:, 132:260]
        w2o_sb = cf_sb[:, 260:388]
        identr_sb = ch_sb[:, 0:128]
        w1a_sb = ch_sb[:, 128:256]
        w1o_sb = ch_sb[:, 256:384]
        va16_sb = ch_sb[:, 384:640].rearrange("a (c k) -> a c k", c=16)
        vo16_sb = ch_sb[:, 640:896].rearrange("a (c k) -> a c k", c=16)

        ones_row = consts.tile([1, 128], F32, tag="ones_row")
        nc.vector.memset(ones_row, 1.0)
        ones_col = consts.tile([128, 1], F32, tag="ones_col")
        nc.vector.memset(ones_col, 1.0)

        # dummy matmul so the PE observes the ch-DMA lane once, up front;
        # walrus allows only one sync wait per (self-loading) Matmult.
        pdum = pp_misc.tile([128, 4], F32, tag="m")
        nc.tensor.matmul(pdum, lhsT=identr_sb, rhs=identr_sb[:, 0:4],
                         start=True, stop=True)
        # bias_a for all local batches: [A, BPC] = W_a2 @ g.T
        pba = pp_misc.tile([A, BPC], F32, tag="m")
        nc.tensor.matmul(pba, lhsT=w2a_sb, rhs=gT_sb, start=True, stop=True)
        ba_sb = consts.tile([A, BPC], F32, tag="ba")
        nc.vector.tensor_copy(out=ba_sb, in_=pba)

        def bcast_scalar(src11, tag):
            """[1,1] sbuf scalar -> [128,1] sbuf per-partition vector."""
            pb = pp_misc.tile([128, 1], F32, tag="m")
            nc.tensor.matmul(pb, lhsT=ones_row, rhs=src11, start=True, stop=True)
            dst = small.tile([128, 1], F32, tag=tag)
            nc.vector.tensor_copy(out=dst, in_=pb)
            return dst

        def score_pass(embT, w1_sb, v16_sb, bias_ap, pat):
            # Returns psc: PSUM [128, NT]; col T=4c+q holds scores of embT
            # col 512c + 128q + p  (== token n = 64p + t, t = 4c+q).
            # The main matmul runs LOOKAHEAD chunks ahead of the v-dot so
            # the PE never stalls on the ACT/DVE relu of the current chunk.
            # psc shares the pp_big ring (allocated after the chunk loop so
            # the ring slots recycle cleanly).
            psc = pp_sc.tile([128, NT], F32, tag="psc")
            srelu = {}
            prow = None

            if VDOT_CT == 2:
                # strip t holds chunks {2j+t}: psc col T = 4(2j+t)+q
                pscv = psc.rearrange("p (j t q) -> p j t q", t=2, q=4)
            else:
                pscv = psc.rearrange("p (pk j q) -> p pk j q", pk=NPACKS, q=4)
            w = 8 if VDOT_CT == 2 else NCH // NPACKS

            def close_pack(pk, rows_src):
                rows_sb = work.tile([w, CH], F32, tag="rows")
                if ROWS_ENG == "A":
                    nc.scalar.activation(out=rows_sb, in_=rows_src, func=COPY)
                else:
                    nc.vector.tensor_copy(out=rows_sb, in_=rows_src)
                for q in range(4 if "tb" not in ablate else 0):
                    dst = (pscv[:, :, pk, q] if VDOT_CT == 2
                           else pscv[:, pk, :, q])
                    nc.tensor.transpose(
                        dst,
                        rows_sb[:, q * 128:(q + 1) * 128],
                        ident_sb[0:w, 0:w],
                    )

            la = 2 * (1 + (LOOKAHEAD > 2)) if RELU_PAIR else LOOKAHEAD
            for c in range(NCH + la):
                if c < NCH and RELU_PAIR:
                    if c % 2 == 0:
                        pxp = pp_big.tile([A, 2 * CH], F32, tag="big")
                    px = pxp[:, (c % 2) * CH:(c % 2 + 1) * CH]
                    if "w" not in ablate:
                        nc.tensor.matmul(
                            px,
                            lhsT=w1_sb,
                            rhs=embT[:, c * CH:(c + 1) * CH],
                            start=True, stop=True,
                        )
                    if c % 2 == 1 and "relu" not in ablate:
                        sr = work.tile([A, 2 * CH], mm_dt, tag="srelu")
                        srelu[c - 1] = sr[:, 0:CH]
                        srelu[c] = sr[:, CH:2 * CH]
                        if pat[(c // 2) % len(pat)] == "D":
                            nc.vector.tensor_scalar(
                                sr, pxp, bias_ap, 0.0, ADD, MAX)
                        else:
                            nc.scalar.activation(out=sr, in_=pxp,
                                                 func=RELU, bias=bias_ap,
                                                 scale=1.0)
                elif c < NCH:
                    px = pp_big.tile([A, CH], F32, tag="big")
                    if "w" not in ablate:
                        nc.tensor.matmul(
                            px,
                            lhsT=w1_sb,
                            rhs=embT[:, c * CH:(c + 1) * CH],
                            start=True, stop=True,
                        )
                    sr = work.tile([A, CH], mm_dt, tag="srelu")
                    srelu[c] = sr
                    if "relu" not in ablate:
                        if pat[c % len(pat)] == "D":
                            nc.vector.tensor_scalar(
                                sr, px, bias_ap, 0.0, ADD, MAX)
                        else:
                            nc.scalar.activation(out=sr, in_=px,
                                                 func=RELU, bias=bias_ap,
                                                 scale=1.0)
                cc = c - la
                if 0 <= cc < NCH and "v" not in ablate:
                    if VDOT_CT == 2:
                        # even/odd chunks on separate PE column-group tiles
                        # (partition bases 0/64) so consecutive v-dots
                        # overlap inside the array.
                        t, j = cc % 2, cc // 2
                        if cc == 0:
                            prow = pp_row.tile([128, CH], F32, tag="row16")
                        lo = cc - j   # 8-window of the [A,16] plane w/ v at j
                        nc.tensor.matmul(
                            prow[64 * t:64 * t + 8, :],
                            lhsT=v16_sb[:, cc, lo:lo + 8],
                            rhs=srelu.pop(cc),
                            start=(j == 0), stop=(j == 7),
                            tile_position=(0, 64 * t),
                            skip_group_check=True,
                        )
                        if cc >= NCH - 2:
                            close_pack(t, prow[64 * t:64 * t + 8, :])
                    else:
                        if cc % w == 0:
                            prow = pp_row.tile([w, CH], F32, tag="row16")
                        # v-dot: stationary [A,w] with column cc%w = v
                        # -> accumulates chunk cc's scores into row cc%w.
                        nc.tensor.matmul(
                            prow,
                            lhsT=v16_sb[:, cc, (cc // w) * w:(cc // w + 1) * w],
                            rhs=srelu.pop(cc),
                            start=(cc % w == 0), stop=(cc % w == w - 1),
                        )
                        if cc % w == w - 1:
                            close_pack(cc // w, prow)
            return psc

        def softmax_stats(sc_ap, tag):
            """Scores are bounded (|s| < ~60 << 88) so fp32 exp without
            max-subtraction is safe; skipping the global max removes six
            serial cross-engine hops per softmax.

            Returns (pexp [128,NT] fp32 sbuf unnormalized exp, recb
            [128,1] sbuf broadcast reciprocal of the global sum)."""
            pexp = work.tile([128, NT], F32, tag="pexp_" + tag)
            rowsum = small.tile([128, 1], F32, tag="rowsum_" + tag)
            nc.scalar.activation(out=pexp, in_=sc_ap, func=EXP,
                                 bias=0.0, scale=1.0, accum_out=rowsum)
            if GPSM:
                # cross-partition sum on the (otherwise idle) Pool engine;
                # the all-reduce output is already broadcast per-partition.
                tot128 = small.tile([128, 1], F32, tag="tot_" + tag)
                nc.gpsimd.partition_all_reduce(tot128, rowsum, channels=128,
                                               reduce_op=bass_isa.ReduceOp.add)
                recb = small.tile([128, 1], F32, tag="recb_" + tag)
                nc.vector.reciprocal(recb, tot128)
                return pexp, recb
            ptot = pp_misc.tile([1, 1], F32, tag="m")
            nc.tensor.matmul(ptot, lhsT=rowsum, rhs=ones_col,
                             start=True, stop=True)
            rec = small.tile([1, 1], F32, tag="rec_" + tag)
            nc.vector.reciprocal(rec, ptot)
            recb = bcast_scalar(rec, "recb_" + tag)
            return pexp, recb

        nb = nbatch or BPC
        state = [dict() for _ in range(nb)]

        def phase_load(b):
            st = state[b]
            nat = big.tile([128, NT, E], mm_dt, tag="nat")
            st["nat"] = nat
            # ~1MB(fp32-read) casting loads via SWDGE; each chunk stays
            # above the ~1MB DMA line-rate knee on the read side. Batch 0
            # uses finer chunks so the first transposes start sooner.
            nch = int(os.environ.get("KERNEL_NCH0", "8")) if b == 0 else int(os.environ.get("KERNEL_NCHB", "4"))
            q = NT // nch
            for h in range(nch):
                nc.gpsimd.dma_start(out=nat[:, h * q:(h + 1) * q, :],
                                    in_=emb_r[b][:, h * q:(h + 1) * q, :])
            mask_sb = maskp.tile([128, NT], F32, tag="mask")
            st["mask"] = mask_sb
            nc.sync.dma_start(out=mask_sb, in_=mask_r[b])

        def phase_trans(b):
            st = state[b]
            nat = st["nat"]
            pdmb = pp_misc.tile([128, 4], F32, tag="m")
            nc.tensor.matmul(pdmb, lhsT=nat[:, 0, :], rhs=nat[:, 0, 0:4],
                             start=True, stop=True)
            embT = big.tile([E, N], mm_dt, tag="embT")
            st["embT"] = embT
            ngr = (NT // tgrp) if "tr" not in ablate else 0
            for g in range(ngr):
                pt = pp_t.tile([128, tgrp * 128], mm_dt, tag="pt")
                for j in range(tgrp):
                    nc.tensor.transpose(pt[:, j * 128:(j + 1) * 128],
                                        nat[:, g * tgrp + j, :], identr_sb)
                dst = embT[:, g * tgrp * 128:(g + 1) * tgrp * 128]
                if EVICT_PAT[g % len(EVICT_PAT)] == "A":
                    nc.scalar.activation(out=dst, in_=pt, func=COPY)
                else:
                    nc.vector.tensor_copy(out=dst, in_=pt)

        def phase_passA(b):
            st = state[b]
            st["psc"] = score_pass(st["embT"], w1a_sb, va16_sb,
                                   ba_sb[:, b:b + 1], RELU_PAT_A)

        def phase_softA(b):
            st = state[b]
            pexp, recb = softmax_stats(st["psc"], "a")
            # normalized fp16 attention weights for the fp16 ctx matmul
            a16 = work.tile([128, NT], MM_DT, tag="a16")
            st["a16"] = a16
            nc.vector.tensor_scalar_mul(a16, pexp, recb)

        def phase_ctx(b):
            st = state[b]
            nat, a16 = st["nat"], st["a16"]
            # context: c[e] = sum_n a_n * emb[n, e].  Grouped 4 tiles per
            # matmul so the moving free dim is 512; the diagonal blocks of
            # the [4, 512] accumulator hold the true partial contexts.
            pc4 = pp_misc.tile([4, 4 * E], F32, tag="m")
            ng = NT // 4 if "ctx" not in ablate else 1
            for g in range(ng):
                nc.tensor.matmul(
                    pc4,
                    lhsT=a16[:, 4 * g:4 * g + 4],
                    rhs=nat[:, 4 * g:4 * g + 4, :].rearrange("p t e -> p (t e)"),
                    start=(g == 0), stop=(g == ng - 1),
                )
            # Accumulator rows live at partitions 0-3; partition-sliced reads
            # must start at 0/32/64/96, so: copy rows to SBUF, PE-transpose
            # each 128-block, pick the diagonal via a stride-5 free-axis AP,
            # reduce into cT [E, 1].
            cd4 = work.tile([4, 4 * E], F32, tag="cd4")
            nc.vector.tensor_copy(out=cd4, in_=pc4)
            ptd = pp_misc.tile([128, 4, 4], F32, tag="m")
            for q in range(4):
                nc.tensor.transpose(ptd[:, q, :], cd4[0:4, q * E:(q + 1) * E],
                                    ident_sb[0:4, 0:4])
            ptd_flat = ptd.rearrange("p a b -> p (a b)")
            diag = bass.AP(tensor=ptd_flat.tensor, offset=ptd_flat.offset,
                           ap=[ptd_flat.ap[0], [5, 4]])
            cT = small.tile([E, 1], F32, tag="cT")
            nc.vector.tensor_reduce(cT, diag, axis=AX_X, op=ADD)
            # bias_o = W_o2 @ c  (a16 already normalized)
            pbo = pp_misc.tile([A, 1], F32, tag="m")
            nc.tensor.matmul(pbo, lhsT=w2o_sb, rhs=cT, start=True, stop=True)
            bo = small.tile([A, 1], F32, tag="bo")
            st["bo"] = bo
            nc.vector.tensor_copy(out=bo, in_=pbo)

        def phase_passO(b):
            st = state[b]
            st["psc2"] = score_pass(st["embT"], w1o_sb, vo16_sb, st["bo"],
                                    RELU_PAT_O)

        def phase_softO(b):
            st = state[b]
            sc2 = work.tile([128, NT], F32, tag="sc2")
            nc.vector.tensor_add(sc2, st["psc2"], st["mask"])
            pexp2, recb2 = softmax_stats(sc2, "o")
            outt = work.tile([128, NT], F32, tag="outt")
            nc.vector.tensor_scalar_mul(outt, pexp2, recb2)
            nc.sync.dma_start(out=out_r[b], in_=outt)

        PIPE = os.environ.get("KERNEL_PIPE", "3")
        for it_ in range(iters):
            if ITERBAR and it_ > 0:
                nc.all_engine_barrier()
            if PIPE == "0":
                for b in range(nb):
                    phase_load(b)
                    phase_trans(b)
                    phase_passA(b)
                    phase_softA(b)
                    phase_ctx(b)
                    phase_passO(b)
                    phase_softO(b)
            elif PIPE == "3":
                phase_load(0)
                phase_trans(0)
                for b in range(nb):
                    if b + 1 < nb:
                        phase_load(b + 1)
                    phase_passA(b)
                    phase_softA(b)
                    phase_ctx(b)
                    phase_passO(b)
                    if b + 1 < nb:
                        phase_trans(b + 1)
                    phase_softO(b)
            elif PIPE == "4":
                # two batches in flight: batch b+1's transpose + pass A are
                # emitted between batch b's ctx and softO so the PE (and the
                # relu engines) stay fed across the serial softmax chains.
                phase_load(0)
                phase_trans(0)
                phase_passA(0)
                for b in range(nb):
                    if b + 1 < nb:
                        phase_load(b + 1)
                    phase_softA(b)
                    phase_ctx(b)
                    if b + 1 < nb:
                        phase_trans(b + 1)
                    phase_passO(b)
                    if b + 1 < nb:
                        phase_passA(b + 1)
                    phase_softO(b)

    return nc


def _fix_multiwait(bir):
    """walrus's PE Matmult codegen accepts a single sync wait. Hoist extra
    waits onto wait-only EventSemaphore instructions inserted just before."""
    n = 0
    for fn in bir["functions"]:
        for bb in fn["blocks"]:
            new = []
            for inst in bb["instructions"]:
                si = inst.get("sync_info") or {}
                w = si.get("on_wait") or []
                if len(w) > 1:
                    for extra in w[:-1]:
                        n += 1
                        new.append({
                            "debug": inst.get("debug", 0),
                            "engine": inst["engine"],
                            "ins": [], "outs": [],
                            "name": f"{inst['name']}-prewait{n}",
                            "opcode": "EventSemaphore",
                            "sync_info": {"on_update": [], "on_wait": [extra]},
                        })
                    si["on_wait"] = [w[-1]]
                new.append(inst)
            bb["instructions"] = new
    return bir


def _patch_serialization(nc):
    import orjson

    orig = nc.to_json_bytes

    def patched(*a, **kw):
        return orjson.dumps(_fix_multiwait(orjson.loads(orig(*a, **kw))))

    nc.to_json_bytes = patched
    return nc


_NC_CACHE = {}


def _get_nc(mm_dt_name=MM_DT_NAME, iters=1):
    key = (mm_dt_name, iters)
    if key not in _NC_CACHE:
        _NC_CACHE[key] = _patch_serialization(
            build(getattr(mybir.dt, mm_dt_name), iters=iters))
    return _NC_CACHE[key]


def _vz16(v):
    z = np.zeros((A, 16, 16), np.float32)
    for c in range(16):
        z[:, c, c] = v
    return z


def _prep_in_maps(inputs, mm_np=None):
    mm_np = mm_np or mybir.dt.np(MM_DT)
    embeddings = np.ascontiguousarray(np.asarray(inputs["embeddings"], np.float32))
    gru = np.asarray(inputs["gru_output"], np.float32).reshape(B, E)
    mask = np.ascontiguousarray(np.asarray(inputs["action_mask"], np.float32))
    W_a = np.asarray(inputs["W_a"], np.float32)
    W_o = np.asarray(inputs["W_o"], np.float32)
    v_a = np.asarray(inputs["v_a"], np.float32)
    v_o = np.asarray(inputs["v_o"], np.float32)

    eye = np.eye(128, dtype=np.float32)
    ch = np.concatenate(
        [eye, W_a[:, :E].T, W_o[:, :E].T,
         _vz16(v_a).reshape(A, 256), _vz16(v_o).reshape(A, 256)], axis=1)
    ch = np.ascontiguousarray(ch).astype(mm_np)

    in_maps = []
    for c in range(NCORES):
        sl = slice(c * BPC, (c + 1) * BPC)
        cf = np.concatenate(
            [gru[sl].T, eye, W_a[:, E:].T, W_o[:, E:].T], axis=1)
        in_maps.append({
            "emb": embeddings[sl],
            "mask": mask[sl],
            "cf": np.ascontiguousarray(cf, np.float32),
            "ch": ch,
        })
    return in_maps


def run(inputs, trace=False):
    nc = _get_nc()
    in_maps = _prep_in_maps(inputs)
    res = run_bass_kernel_spmd(nc, in_maps, core_ids=list(range(NCORES)),
                               trace=trace)
    out = np.concatenate([res.results[c]["out"] for c in range(NCORES)], axis=0)
    return out.reshape(B, N), res


def kernel(**inputs):
    out, _ = run(inputs, trace=False)
    return out


def make_runner(mm_dt_name=MM_DT_NAME, iters=1):
    """Build the sharded PJRT callable once, for repeated timed execution.

    Mirrors the multi-core branch of bass2jax.run_bass_via_pjrt."""
    import jax
    from jax.experimental.shard_map import shard_map
    from jax.sharding import Mesh, PartitionSpec

    from concourse import bass2jax as b2j
    from concourse import mybir as _mybir

    b2j.install_neuronx_cc_hook()
    nc = _get_nc(mm_dt_name, iters=iters)

    partition_name = (nc.partition_id_tensor.name
                      if nc.partition_id_tensor else None)
    in_names, out_names, out_avals, zero_outs = [], [], [], []
    for alloc in nc.m.functions[0].allocations:
        if not isinstance(alloc, _mybir.MemoryLocationSet):
            continue
        name = alloc.memorylocations[0].name
        if alloc.kind == "ExternalInput":
            if name != partition_name:
                in_names.append(name)
        elif alloc.kind == "ExternalOutput":
            out_names.append(name)
            shape = tuple(alloc.tensor_shape)
            dtype = _mybir.dt.np(alloc.dtype)
            out_avals.append(jax.core.ShapedArray(shape, dtype))
            zero_outs.append(np.zeros(shape, dtype))
    n_params = len(in_names)
    n_outs = len(out_avals)
    all_names = in_names + out_names
    if partition_name is not None:
        all_names = all_names + [partition_name]

    def _body(*args):
        operands = list(args)
        if partition_name is not None:
            operands.append(b2j.partition_id_tensor())
        outs = b2j._bass_exec_p.bind(
            *operands,
            out_avals=tuple(out_avals),
            in_names=tuple(all_names),
            out_names=tuple(out_names),
            lowering_input_output_aliases=(),
            sim_require_finite=True,
            sim_require_nnan=True,
            nc=nc,
        )
        return tuple(outs)

    devices = jax.devices()[:NCORES]
    mesh = Mesh(np.asarray(devices), ("core",))
    donate = tuple(range(n_params, n_params + n_outs))
    sharded = jax.jit(
        shard_map(_body, mesh=mesh,
                  in_specs=(PartitionSpec("core"),) * (n_params + n_outs),
                  out_specs=(PartitionSpec("core"),) * n_outs,
                  check_rep=False),
        donate_argnums=donate, keep_unused=True,
    )

    def runner(inputs, iters=10, burst=True):
        import time as _time
        in_maps = _prep_in_maps(inputs, mm_np=_mybir.dt.np(
            getattr(_mybir.dt, mm_dt_name)))
        concat_in = [
            np.concatenate([np.asarray(in_maps[c][nm]) for c in range(NCORES)], axis=0)
            for nm in in_names
        ]
        concat_in = [jax.device_put(x) for x in concat_in]
        for x in concat_in:
            x.block_until_ready()

        def zeros():
            return [np.zeros((NCORES * z.shape[0], *z.shape[1:]), z.dtype)
                    for z in zero_outs]

        out = sharded(*concat_in, *zeros())  # warm / compile
        [o.block_until_ready() for o in out]
        result = np.asarray(out[0]).reshape(B, N)

        seq_times = []
        for _ in range(iters):
            zs = zeros()
            t0 = _time.perf_counter()
            out = sharded(*concat_in, *zs)
            [o.block_until_ready() for o in out]
            seq_times.append(_time.perf_counter() - t0)

        zss = [zeros() for _ in range(iters)]
        t0 = _time.perf_counter()
        outs = [sharded(*concat_in, *zs) for zs in zss]
        [o.block_until_ready() for o in outs[-1]]
        burst_time = (_time.perf_counter() - t0) / iters
        return result, {
            "seq_min_s": min(seq_times),
            "seq_med_s": sorted(seq_times)[len(seq_times) // 2],
            "burst_avg_s": burst_time,
        }

    return runner

